# revision 19
# baseline (speedup 1.0000x reference)
"""MoE (8 experts, top-2) Trainium2 kernel — class-rebalanced expert-parallel.

Each core runs three uniform streams over token tiles gathered on host:
- F-stream (5 tiles, one class-0/2 expert): up+gelu -> M1 [F,F] -> LN-blend
  +gelu -> h2 = B (transposed back in place)
- H-stream (3 tiles, one class-1 expert): up+gelu -> M1h [F,F2]+gelu -> M2
  [F2,F] -> LN -> h2 = LN(R)
- I-stream (2 tiles, one class-3 expert): up+gelu -> h2 = h1
Then down [F,H] per stream + combine-weight scale. Host splits each expert's
token tiles across exactly two cores (one expert per stream per core), so no
cross-core traffic; pads are zero-weight tiles. All matmul operands bf16;
LN/router replicated on host (pre-normalized tokens shipped transposed).
"""

import os
import numpy as np
import ml_dtypes

BF16 = ml_dtypes.bfloat16

B, S, H, F, E, K = 2, 2048, 1024, 4096, 8, 2
F2 = F // 2
T = B * S
P = 128
NTF, NTH, NTI = 5, 3, 2
NTT = NTF + NTH + NTI          # 10 tile slots per core
FB, HB, IB = 0, NTF * P, (NTF + NTH) * P   # token-slot bases: 0, 640, 1024
TOT = NTT * P                  # 1280
KH = H // P                    # 8
MF = F // P                    # 32
K2 = F2 // P                   # 16
NF = F // 512                  # 8
N4 = F2 // 512                 # 4
ND = H // 512                  # 2
EPS = 1e-5

F_EXPERTS = [0, 4, 2, 6]       # class0: 0,4 (al1=1); class2: 2,6 (al1=0)
H_EXPERTS = [1, 5]
I_EXPERTS = [3, 7]

_CACHED_NC = None


def _build_nc():
    import concourse.mybir as mybir
    import concourse.tile as tile
    from concourse import bacc

    f32, AF = mybir.dt.float32, mybir.ActivationFunctionType
    bf16 = mybir.dt.bfloat16
    ALU = mybir.AluOpType
    nc = bacc.Bacc(num_devices=8)

    nxt_e = nc.declare_dram_parameter("nxt", [P, KH, TOT], bf16, isOutput=False)
    idb_e = nc.declare_dram_parameter("idb", [P, P], bf16, isOutput=False)
    wv_e = nc.declare_dram_parameter("wv", [P, NTT], f32, isOutput=False)
    al1_e = nc.declare_dram_parameter("al1", [P, 1], f32, isOutput=False)
    upwF_e = nc.declare_dram_parameter("upwF", [MF, P, KH, P], bf16, isOutput=False)
    upwH_e = nc.declare_dram_parameter("upwH", [MF, P, KH, P], bf16, isOutput=False)
    upwI_e = nc.declare_dram_parameter("upwI", [MF, P, KH, P], bf16, isOutput=False)
    upbF_e = nc.declare_dram_parameter("upbF", [P, MF], f32, isOutput=False)
    upbH_e = nc.declare_dram_parameter("upbH", [P, MF], f32, isOutput=False)
    upbI_e = nc.declare_dram_parameter("upbI", [P, MF], f32, isOutput=False)
    w1f_e = nc.declare_dram_parameter("w1f", [NF, MF, P, 512], bf16, isOutput=False)
    w1h_e = nc.declare_dram_parameter("w1h", [N4, MF, P, 512], bf16, isOutput=False)
    w2h_e = nc.declare_dram_parameter("w2h", [NF, K2, P, 512], bf16, isOutput=False)
    dwF_e = nc.declare_dram_parameter("dwF", [ND, MF, P, 512], bf16, isOutput=False)
    dwH_e = nc.declare_dram_parameter("dwH", [MF, P, H], bf16, isOutput=False)
    dwI_e = nc.declare_dram_parameter("dwI", [MF, P, H], bf16, isOutput=False)
    y_e = nc.declare_dram_parameter("y", [TOT, H], f32, isOutput=True)

    with tile.TileContext(nc) as tc:
        with tc.tile_pool(name="cst", bufs=1) as cst, \
             tc.tile_pool(name="sb", bufs=4) as sb, \
             tc.tile_pool(name="stat", bufs=6) as stp, \
             tc.tile_pool(name="slab", bufs=10) as slp, \
             tc.tile_pool(name="bigH", bufs=1) as bigH, \
             tc.tile_pool(name="bigN", bufs=1) as bigN, \
             tc.tile_pool(name="bigA", bufs=1) as bigA, \
             tc.tile_pool(name="bigAH", bufs=1) as bigAH, \
             tc.tile_pool(name="upslp", bufs=3) as upslp, \
             tc.tile_pool(name="dwslp", bufs=4) as dwslp, \
             tc.tile_pool(name="psb", bufs=8, space="PSUM") as psb:

            identb = cst.tile([P, P], bf16)
            wv = cst.tile([P, NTT], f32)
            al1 = cst.tile([P, 1], f32)
            upbF = cst.tile([P, MF], f32)
            upbH = cst.tile([P, MF], f32)
            upbI = cst.tile([P, MF], f32)
            eps_t = cst.tile([P, 1], f32)
            nc.vector.memset(eps_t[:], EPS)

            def load_consts():
                nc.sync.dma_start(out=upbF[:], in_=upbF_e.ap())
                nc.sync.dma_start(out=upbH[:], in_=upbH_e.ap())
                nc.sync.dma_start(out=upbI[:], in_=upbI_e.ap())
                nc.sync.dma_start(out=identb[:], in_=idb_e.ap())
                nc.sync.dma_start(out=wv[:], in_=wv_e.ap())
                nc.sync.dma_start(out=al1[:], in_=al1_e.ap())

            def ln_coeffs(src_stats, alpha, negshift_out, scale_out):
                """src_stats [P,2] (mean,var) -> scale=a*rstd+(1-a),
                shift=-a*mean*rstd (per-partition).  alpha: AP or 1.0."""
                rstd = stp.tile([P, 1], f32, tag="rstd")
                nc.scalar.activation(out=rstd[:], in_=src_stats[:, 1:2],
                                     func=AF.Sqrt, bias=eps_t[:, 0:1])
                nc.vector.reciprocal(out=rstd[:], in_=rstd[:])
                if alpha is None:          # alpha == 1: pure LN
                    nc.vector.tensor_copy(out=scale_out[:], in_=rstd[:])
                    nc.vector.tensor_tensor(out=negshift_out[:],
                                            in0=src_stats[:, 0:1], in1=rstd[:],
                                            op=ALU.mult)
                    nc.vector.tensor_scalar(out=negshift_out[:], in0=negshift_out[:],
                                            scalar1=-1.0, scalar2=None, op0=ALU.mult)
                    return
                nc.vector.tensor_scalar(out=scale_out[:], in0=rstd[:],
                                        scalar1=alpha, scalar2=None, op0=ALU.mult)
                one_m = stp.tile([P, 1], f32, tag="onem")
                nc.vector.tensor_scalar(out=one_m[:], in0=alpha, scalar1=-1.0,
                                        scalar2=1.0, op0=ALU.mult, op1=ALU.add)
                nc.vector.tensor_tensor(out=scale_out[:], in0=scale_out[:],
                                        in1=one_m[:], op=ALU.add)
                nc.vector.tensor_tensor(out=negshift_out[:], in0=src_stats[:, 0:1],
                                        in1=rstd[:], op=ALU.mult)
                nc.vector.tensor_scalar(out=negshift_out[:], in0=negshift_out[:],
                                        scalar1=-1.0, scalar2=None, op0=ALU.mult)
                nc.vector.tensor_tensor(out=negshift_out[:], in0=negshift_out[:],
                                        in1=alpha, op=ALU.mult)

            # ---------- tokens (pre-normalized, transposed) ----------
            nxT = bigN.tile([P, KH, TOT], bf16, tag="nxT")
            nc.sync.dma_start(out=nxT[:, :, FB:HB], in_=nxt_e.ap()[:, :, FB:HB])

            # ---------- up: h1T = gelu(upW.T @ nxT + upb), all 3 streams ----------
            h1T = bigH.tile([P, MF, TOT], bf16, tag="h1T")
            UP_GRPS = [(0, FB, NTF * P // 2, upbF), (0, FB + NTF * P // 2,
                        NTF * P // 2, upbF), (1, HB, NTH * P, upbH),
                       (2, IB, NTI * P, upbI)]
            UP_WES = [upwF_e, upwH_e, upwI_e]
            for m in range(MF):
                us = upslp.tile([P, 3, KH, P], bf16, tag="uslab")
                for si in range(3):
                    nc.sync.dma_start(out=us[:, si], in_=UP_WES[si].ap()[m])
                if m == 0:
                    load_consts()
                    nc.sync.dma_start(out=nxT[:, :, HB:IB], in_=nxt_e.ap()[:, :, HB:IB])
                    nc.sync.dma_start(out=nxT[:, :, IB:TOT], in_=nxt_e.ap()[:, :, IB:TOT])
                ps_g = [psb.tile([P, G], f32, space="PSUM", tag="acc",
                                 name=f"upps_{m}_{gi}")
                        for gi, (si, c0, G, upb) in enumerate(UP_GRPS)]
                for k in range(KH):
                    for gi, (si, c0, G, upb) in enumerate(UP_GRPS):
                        nc.tensor.matmul(out=ps_g[gi][:], lhsT=us[:, si, k, :],
                                         rhs=nxT[:, k, c0:c0 + G],
                                         start=(k == 0), stop=(k == KH - 1))
                for gi, (si, c0, G, upb) in enumerate(UP_GRPS):
                    nc.scalar.activation(out=h1T[:, m, c0:c0 + G], in_=ps_g[gi][:],
                                         func=AF.Gelu, bias=upb[:, m:m + 1])

            # ---------- M1f: Af = h1F @ W1f + b1f  (SBUF, bf16) + stats ----------
            AbufF = bigA.tile([P, NTF, F], bf16, tag="AbufF")
            stA = [stp.tile([P, NF, nc.vector.BN_STATS_DIM], f32, tag=f"stA{t}",
                            name=f"stA_{t}") for t in range(NTF)]
            for n in range(NF):
                ps_list = [psb.tile([P, 512], f32, space="PSUM", tag="acc",
                                    name=f"accf_{n}_{i}") for i in range(NTF)]
                for k in range(MF):
                    wslab = slp.tile([P, 512], bf16, tag="wslab")
                    nc.sync.dma_start(out=wslab[:], in_=w1f_e.ap()[n, k])
                    for t in range(NTF):
                        nc.tensor.matmul(out=ps_list[t][:],
                                         lhsT=h1T[:, k, FB + t * P:FB + (t + 1) * P],
                                         rhs=wslab[:], start=(k == 0), stop=(k == MF - 1))
                for t in range(NTF):
                    nc.vector.tensor_copy(out=AbufF[:, t, n * 512:(n + 1) * 512],
                                          in_=ps_list[t][:])
                for t in range(NTF):
                    nc.vector.bn_stats(out=stA[t][:, n, :],
                                       in_=AbufF[:, t, n * 512:(n + 1) * 512])

            # ---------- M1h: Ah = gelu(h1H @ W1h + b1h)  [F2 wide] ----------
            AbufH = bigAH.tile([P, NTH, F2], bf16, tag="AbufH")
            for n in range(N4):
                ps_list = [psb.tile([P, 512], f32, space="PSUM", tag="acc",
                                    name=f"acch_{n}_{i}") for i in range(NTH)]
                for k in range(MF):
                    wslab = slp.tile([P, 512], bf16, tag="wslab")
                    nc.sync.dma_start(out=wslab[:], in_=w1h_e.ap()[n, k])
                    for t in range(NTH):
                        nc.tensor.matmul(out=ps_list[t][:],
                                         lhsT=h1T[:, k, HB + t * P:HB + (t + 1) * P],
                                         rhs=wslab[:], start=(k == 0), stop=(k == MF - 1))
                for t in range(NTH):
                    nc.scalar.activation(out=AbufH[:, t, n * 512:(n + 1) * 512],
                                         in_=ps_list[t][:], func=AF.Gelu)

            # ---------- sigma1f: B = gelu(LN-blend(Af)); h2_F = B.T ----------
            for t in range(NTF):
                mvA = stp.tile([P, nc.vector.BN_AGGR_DIM], f32, tag="mvA")
                nc.vector.bn_aggr(out=mvA[:], in_=stA[t][:])
                sc1 = stp.tile([P, 1], f32, tag="sc1")
                sh1 = stp.tile([P, 1], f32, tag="sh1")
                ln_coeffs(mvA, al1[:, 0:1], sh1, sc1)
                for s in range(2):
                    nc.scalar.activation(out=AbufF[:, t, s * 2048:(s + 1) * 2048],
                                         in_=AbufF[:, t, s * 2048:(s + 1) * 2048],
                                         func=AF.Gelu, bias=sh1[:, 0:1],
                                         scale=sc1[:, 0:1])
                for k in range(MF):
                    tpb = psb.tile([P, P], bf16, space="PSUM", tag="acc", name=f"tpb_{t}_{k}")
                    nc.tensor.transpose(out=tpb[:],
                                        in_=AbufF[:, t, k * P:(k + 1) * P],
                                        identity=identb[:])
                    if k % 2 == 0:
                        nc.vector.tensor_copy(out=h1T[:, k, FB + t * P:FB + (t + 1) * P],
                                              in_=tpb[:])
                    else:
                        nc.scalar.activation(out=h1T[:, k, FB + t * P:FB + (t + 1) * P],
                                             in_=tpb[:], func=AF.Copy)

            # ---------- CT = Ah.T ----------
            CT = bigN.tile([P, K2, NTH * P], bf16, tag="nxT")  # reuse nxT slot
            for t in range(NTH):
                for kk in range(K2):
                    tp = psb.tile([P, P], bf16, space="PSUM", tag="acc", name=f"tpc_{t}_{kk}")
                    nc.tensor.transpose(out=tp[:],
                                        in_=AbufH[:, t, kk * P:(kk + 1) * P],
                                        identity=identb[:])
                    if kk % 2 == 0:
                        nc.vector.tensor_copy(out=CT[:, kk, t * P:(t + 1) * P], in_=tp[:])
                    else:
                        nc.scalar.activation(out=CT[:, kk, t * P:(t + 1) * P],
                                             in_=tp[:], func=AF.Copy)

            # ---------- M2: R = Ah @ W2h + b2h  + stats ----------
            RbufH = bigA.tile([P, NTH, F], bf16, tag="AbufF")   # reuse AbufF slot
            st2 = [stp.tile([P, NF, nc.vector.BN_STATS_DIM], f32, tag=f"st2{t}",
                            name=f"st2_{t}") for t in range(NTH)]
            for n in range(NF):
                ps_list = [psb.tile([P, 512], f32, space="PSUM", tag="acc",
                                    name=f"acc2_{n}_{i}") for i in range(NTH)]
                for kk in range(K2):
                    wslab = slp.tile([P, 512], bf16, tag="wslab")
                    nc.sync.dma_start(out=wslab[:], in_=w2h_e.ap()[n, kk])
                    for t in range(NTH):
                        nc.tensor.matmul(out=ps_list[t][:],
                                         lhsT=CT[:, kk, t * P:(t + 1) * P],
                                         rhs=wslab[:], start=(kk == 0), stop=(kk == K2 - 1))
                for t in range(NTH):
                    nc.scalar.activation(out=RbufH[:, t, n * 512:(n + 1) * 512],
                                         in_=ps_list[t][:], func=AF.Copy)
                    nc.vector.bn_stats(out=st2[t][:, n, :],
                                       in_=RbufH[:, t, n * 512:(n + 1) * 512])

            # ---------- down F (h2 ready after sigma1f) ----------
            def down_pass(base, ntx, dw_e):
                for n in range(ND):
                    ps_list = [psb.tile([P, 512], f32, space="PSUM", tag="acc",
                                        name=f"accd_{base}_{n}_{i}") for i in range(ntx)]
                    for k in range(MF):
                        dslab = slp.tile([P, 512], bf16, tag="wslab")
                        nc.sync.dma_start(out=dslab[:], in_=dw_e.ap()[n, k])
                        for t in range(ntx):
                            nc.tensor.matmul(out=ps_list[t][:],
                                             lhsT=h1T[:, k, base + t * P:base + (t + 1) * P],
                                             rhs=dslab[:], start=(k == 0), stop=(k == MF - 1))
                    for t in range(ntx):
                        slot = base // P + t
                        yv = sb.tile([P, 512], f32, tag="yv")
                        nc.scalar.activation(out=yv[:], in_=ps_list[t][:], func=AF.Copy,
                                             scale=wv[:, slot:slot + 1])
                        nc.sync.dma_start(out=y_e.ap()[slot * P:(slot + 1) * P,
                                                       n * 512:(n + 1) * 512],
                                          in_=yv[:])

            def down_pass2(base, ntx, dw_e):
                ps = [[psb.tile([P, 512], f32, space="PSUM", tag="acc",
                                name=f"accd2_{base}_{t}_{n}") for n in range(ND)]
                      for t in range(ntx)]
                for k in range(MF):
                    dslab = dwslp.tile([P, H], bf16, tag="dwslab")
                    nc.sync.dma_start(out=dslab[:], in_=dw_e.ap()[k])
                    for t in range(ntx):
                        for n in range(ND):
                            nc.tensor.matmul(out=ps[t][n][:],
                                             lhsT=h1T[:, k, base + t * P:base + (t + 1) * P],
                                             rhs=dslab[:, n * 512:(n + 1) * 512],
                                             start=(k == 0), stop=(k == MF - 1))
                for t in range(ntx):
                    slot = base // P + t
                    for n in range(ND):
                        yv = sb.tile([P, 512], f32, tag="yv")
                        nc.scalar.activation(out=yv[:], in_=ps[t][n][:], func=AF.Copy,
                                             scale=wv[:, slot:slot + 1])
                        nc.sync.dma_start(out=y_e.ap()[slot * P:(slot + 1) * P,
                                                       n * 512:(n + 1) * 512],
                                          in_=yv[:])

            down_pass(FB, NTF, dwF_e)
            down_pass2(IB, NTI, dwI_e)

            # ---------- sigma2: h2_H = LN(R).T ----------
            for t in range(NTH):
                mv2 = stp.tile([P, nc.vector.BN_AGGR_DIM], f32, tag="mv2")
                nc.vector.bn_aggr(out=mv2[:], in_=st2[t][:])
                sc2 = stp.tile([P, 1], f32, tag="sc2")
                sh2 = stp.tile([P, 1], f32, tag="sh2")
                ln_coeffs(mv2, None, sh2, sc2)      # alpha = 1 (pure LN)
                for s in range(2):
                    nc.vector.tensor_scalar(out=RbufH[:, t, s * 2048:(s + 1) * 2048],
                                            in0=RbufH[:, t, s * 2048:(s + 1) * 2048],
                                            scalar1=sc2[:, 0:1], scalar2=sh2[:, 0:1],
                                            op0=ALU.mult, op1=ALU.add)
                for k in range(MF):
                    tpr = psb.tile([P, P], bf16, space="PSUM", tag="acc", name=f"tpr_{t}_{k}")
                    nc.tensor.transpose(out=tpr[:],
                                        in_=RbufH[:, t, k * P:(k + 1) * P],
                                        identity=identb[:])
                    if k % 2 == 0:
                        nc.vector.tensor_copy(out=h1T[:, k, HB + t * P:HB + (t + 1) * P],
                                              in_=tpr[:])
                    else:
                        nc.scalar.activation(out=h1T[:, k, HB + t * P:HB + (t + 1) * P],
                                             in_=tpr[:], func=AF.Copy)

            # ---------- down H ----------
            down_pass2(HB, NTH, dwH_e)

    nc.finalize()
    return nc


def _routing(x_flat, ln_g, ln_b, router_w):
    """Bit-exact replication of the reference router on jax CPU.
    Returns (combine weights, top-k mask, normalized tokens)."""
    import jax
    import jax.numpy as jnp
    cpu = jax.devices("cpu")[0]
    with jax.default_device(cpu):
        x = jnp.asarray(np.asarray(x_flat))
        g = jnp.asarray(np.asarray(ln_g))
        b = jnp.asarray(np.asarray(ln_b))
        rw = jnp.asarray(np.asarray(router_w))
        m = jnp.mean(x, axis=-1, keepdims=True)
        v = jnp.var(x, axis=-1, keepdims=True)
        nx = (x - m) / jnp.sqrt(v + 1e-5) * g + b
        logits = nx @ rw
        probs = jax.nn.softmax(logits, axis=-1)
        _, idx = jax.lax.top_k(probs, K)
        mask = jnp.sum(jax.nn.one_hot(idx, probs.shape[-1], dtype=probs.dtype), axis=1)
        w = probs * mask
        w = w / jnp.sum(w, axis=-1, keepdims=True)
        return np.asarray(w), np.asarray(mask), np.asarray(nx)


def _col128(vec, n):
    return np.ascontiguousarray(vec.reshape(n, P).T)


def _swz(wmat, nf, kk):
    # [kk*128, nf*512] -> [nf, kk, 128, 512]
    r = wmat.reshape(kk, P, nf, 512)
    return np.ascontiguousarray(r.transpose(2, 0, 1, 3)).astype(BF16)


def _chunks(lst, size):
    return [lst[i:i + size] for i in range(0, len(lst), size)]


def kernel(**inputs):
    from concourse.bass_utils import run_bass_kernel_spmd

    global _CACHED_NC
    x = np.asarray(inputs["hidden_states"], np.float32)
    x_flat = x.reshape(T, H)
    w_all, mask, nx = _routing(x_flat, inputs["ln_g"], inputs["ln_b"],
                               inputs["router_w"])

    up_W = np.asarray(inputs["up_W"], np.float32)
    up_b = np.asarray(inputs["up_b"], np.float32)
    down_W = np.asarray(inputs["down_W"], np.float32)
    down_b = np.asarray(inputs["down_b"], np.float32)
    spec0_W = np.asarray(inputs["spec0_W"], np.float32)
    spec0_b = np.asarray(inputs["spec0_b"], np.float32)
    spec1a_W = np.asarray(inputs["spec1a_W"], np.float32)
    spec1a_b = np.asarray(inputs["spec1a_b"], np.float32)
    spec1b_W = np.asarray(inputs["spec1b_W"], np.float32)
    spec1b_b = np.asarray(inputs["spec1b_b"], np.float32)
    spec2_W = np.asarray(inputs["spec2_W"], np.float32)
    spec2_b = np.asarray(inputs["spec2_b"], np.float32)
    ln0_g = np.asarray(inputs["ln0_g"], np.float32)
    ln0_b = np.asarray(inputs["ln0_b"], np.float32)
    ln1_g = np.asarray(inputs["ln1_g"], np.float32)
    ln1_b = np.asarray(inputs["ln1_b"], np.float32)
    assert np.all(ln0_g == 1) and np.all(ln0_b == 0), "ln0 affine folding not implemented"
    assert np.all(ln1_g == 1) and np.all(ln1_b == 0), "ln1 affine folding not implemented"
    for bias in (up_b, down_b, spec0_b, spec1a_b, spec1b_b, spec2_b):
        assert np.all(bias == 0), "nonzero biases not supported by this kernel"

    # --- per-expert padded token-tile lists ---
    tok = {e: np.nonzero(mask[:, e] > 0)[0] for e in range(E)}
    tiles = {}          # e -> list of arrays of 128 token ids (padded with -1)
    for e in range(E):
        ids = tok[e]
        ntile = (len(ids) + P - 1) // P
        pad = np.full(ntile * P, -1, np.int64)
        pad[:len(ids)] = ids
        tiles[e] = [pad[i * P:(i + 1) * P] for i in range(ntile)]

    # --- assign tile chunks to cores: one expert per stream per core ---
    def assign(experts, cap):
        per_core = [None] * 8       # (expert, [tiles]) or None
        c = 0
        for e in experts:
            for ch in _chunks(tiles[e], cap):
                assert c < 8, "stream assignment overflow"
                per_core[c] = (e, ch)
                c += 1
        return per_core

    f_asgn = assign(F_EXPERTS, NTF)
    h_asgn = assign(H_EXPERTS, NTH)
    i_asgn = assign(I_EXPERTS, NTI)
    assert all(a is not None for a in f_asgn), "F stream must fill all cores"

    zeros_w1f = np.zeros((NF, MF, P, 512), BF16)
    zeros_w1h = np.zeros((N4, MF, P, 512), BF16)
    zeros_w2h = np.zeros((NF, K2, P, 512), BF16)
    zeros_dw = np.zeros((ND, MF, P, 512), BF16)
    zeros_dw2 = np.zeros((MF, P, H), BF16)
    zeros_upw = np.zeros((MF, P, KH, P), BF16)

    def upw_swz(e):
        return np.ascontiguousarray(
            up_W[e].reshape(KH, P, MF, P).transpose(2, 1, 0, 3)).astype(BF16)

    in_maps = []
    scatter = []        # per core: (slot_token_ids [TOT], valid mask [TOT])
    for c in range(8):
        slot_ids = np.zeros(TOT, np.int64)
        valid = np.zeros(TOT, bool)
        wvv = np.zeros(TOT, np.float32)

        def fill(asgn, base, nslots):
            if asgn is None:
                return None
            e, ch = asgn
            for ti, tile_ids in enumerate(ch):
                s0 = base + ti * P
                real = tile_ids >= 0
                slot_ids[s0:s0 + P][real] = tile_ids[real]
                valid[s0:s0 + P] = real
                wvv[s0:s0 + P][real] = w_all[tile_ids[real], e]
            return e

        eF = fill(f_asgn[c], FB, NTF)
        eH = fill(h_asgn[c], HB, NTH)
        eI = fill(i_asgn[c], IB, NTI)

        nxg = np.zeros((TOT, H), np.float32)
        nxg[valid] = nx[slot_ids[valid]]
        nxt = np.ascontiguousarray(
            nxg.reshape(TOT, KH, P).transpose(2, 1, 0)).astype(BF16)

        jF = 0 if eF < 4 else 1
        if eF in (0, 4):
            w1f, b1f, a1 = spec0_W[jF], spec0_b[jF], 1.0
        else:
            w1f, b1f, a1 = spec2_W[jF], spec2_b[jF], 0.0

        im = {
            "nxt": nxt,
            "idb": np.eye(P, dtype=np.float32).astype(BF16),
            "wv": _col128(wvv, NTT),
            "al1": np.full((P, 1), a1, np.float32),
            "upwF": upw_swz(eF),
            "upbF": _col128(up_b[eF], MF),
            "w1f": _swz(w1f, NF, MF),
            "dwF": _swz(down_W[eF], ND, MF),
        }
        if eH is not None:
            jH = 0 if eH < 4 else 1
            im["upwH"] = upw_swz(eH)
            im["upbH"] = _col128(up_b[eH], MF)
            im["w1h"] = _swz(spec1a_W[jH], N4, MF)
            im["w2h"] = _swz(spec1b_W[jH], NF, K2)
            im["dwH"] = np.ascontiguousarray(down_W[eH].reshape(MF, P, H)).astype(BF16)
        else:
            im["upwH"] = zeros_upw
            im["upbH"] = np.zeros((P, MF), np.float32)
            im["w1h"] = zeros_w1h
            im["w2h"] = zeros_w2h
            im["dwH"] = zeros_dw2
        if eI is not None:
            im["upwI"] = upw_swz(eI)
            im["upbI"] = _col128(up_b[eI], MF)
            im["dwI"] = np.ascontiguousarray(down_W[eI].reshape(MF, P, H)).astype(BF16)
        else:
            im["upwI"] = zeros_upw
            im["upbI"] = np.zeros((P, MF), np.float32)
            im["dwI"] = zeros_dw2

        in_maps.append(im)
        scatter.append((slot_ids, valid))

    if _CACHED_NC is None:
        _CACHED_NC = _build_nc()
    trace = os.environ.get("BASS_MOE_TRACE") == "1"
    res = run_bass_kernel_spmd(_CACHED_NC, in_maps, list(range(8)), trace=trace)
    global LAST_RES
    LAST_RES = res

    y = x_flat.copy()
    for c in range(8):
        slot_ids, valid = scatter[c]
        out = res.results[c]["y"]
        np.add.at(y, slot_ids[valid], out[valid])
    return y.reshape(B, S, H)


# revision 21
# speedup vs baseline: 1.0032x; 1.0032x over previous
"""MoE (8 experts, top-2) Trainium2 kernel — class-rebalanced expert-parallel.

Each core runs three uniform streams over token tiles gathered on host:
- F-stream (5 tiles, one class-0/2 expert): up+gelu -> M1 [F,F] -> LN-blend
  +gelu -> h2 = B (transposed back in place)
- H-stream (3 tiles, one class-1 expert): up+gelu -> M1h [F,F2]+gelu -> M2
  [F2,F] -> LN -> h2 = LN(R)
- I-stream (2 tiles, one class-3 expert): up+gelu -> h2 = h1
Then down [F,H] per stream + combine-weight scale. Host splits each expert's
token tiles across exactly two cores (one expert per stream per core), so no
cross-core traffic; pads are zero-weight tiles. All matmul operands bf16;
LN/router replicated on host (pre-normalized tokens shipped transposed).
"""

import os
import numpy as np
import ml_dtypes

BF16 = ml_dtypes.bfloat16

B, S, H, F, E, K = 2, 2048, 1024, 4096, 8, 2
F2 = F // 2
T = B * S
P = 128
NTF, NTH, NTI = 5, 3, 2
NTT = NTF + NTH + NTI          # 10 tile slots per core
FB, HB, IB = 0, NTF * P, (NTF + NTH) * P   # token-slot bases: 0, 640, 1024
TOT = NTT * P                  # 1280
KH = H // P                    # 8
MF = F // P                    # 32
K2 = F2 // P                   # 16
NF = F // 512                  # 8
N4 = F2 // 512                 # 4
ND = H // 512                  # 2
EPS = 1e-5

F_EXPERTS = [0, 4, 2, 6]       # class0: 0,4 (al1=1); class2: 2,6 (al1=0)
H_EXPERTS = [1, 5]
I_EXPERTS = [3, 7]

_CACHED_NC = None


def _build_nc():
    import concourse.mybir as mybir
    import concourse.tile as tile
    from concourse import bacc

    f32, AF = mybir.dt.float32, mybir.ActivationFunctionType
    bf16 = mybir.dt.bfloat16
    ALU = mybir.AluOpType
    nc = bacc.Bacc(num_devices=8)

    nxt_e = nc.declare_dram_parameter("nxt", [P, KH, TOT], bf16, isOutput=False)
    idb_e = nc.declare_dram_parameter("idb", [P, P], bf16, isOutput=False)
    wv_e = nc.declare_dram_parameter("wv", [P, NTT], f32, isOutput=False)
    al1_e = nc.declare_dram_parameter("al1", [P, 1], f32, isOutput=False)
    upwF_e = nc.declare_dram_parameter("upwF", [MF, P, KH, P], bf16, isOutput=False)
    upwH_e = nc.declare_dram_parameter("upwH", [MF, P, KH, P], bf16, isOutput=False)
    upwI_e = nc.declare_dram_parameter("upwI", [MF, P, KH, P], bf16, isOutput=False)
    upbF_e = nc.declare_dram_parameter("upbF", [P, MF], f32, isOutput=False)
    upbH_e = nc.declare_dram_parameter("upbH", [P, MF], f32, isOutput=False)
    upbI_e = nc.declare_dram_parameter("upbI", [P, MF], f32, isOutput=False)
    w1f_e = nc.declare_dram_parameter("w1f", [NF, MF, P, 512], bf16, isOutput=False)
    w1h_e = nc.declare_dram_parameter("w1h", [N4, MF, P, 512], bf16, isOutput=False)
    w2h_e = nc.declare_dram_parameter("w2h", [NF, K2, P, 512], bf16, isOutput=False)
    dwF_e = nc.declare_dram_parameter("dwF", [ND, MF, P, 512], bf16, isOutput=False)
    dwH_e = nc.declare_dram_parameter("dwH", [MF, P, H], bf16, isOutput=False)
    dwI_e = nc.declare_dram_parameter("dwI", [MF, P, H], bf16, isOutput=False)
    y_e = nc.declare_dram_parameter("y", [TOT, H], f32, isOutput=True)

    with tile.TileContext(nc) as tc:
        with tc.tile_pool(name="cst", bufs=1) as cst, \
             tc.tile_pool(name="sb", bufs=4) as sb, \
             tc.tile_pool(name="stat", bufs=6) as stp, \
             tc.tile_pool(name="slab", bufs=10) as slp, \
             tc.tile_pool(name="bigH", bufs=1) as bigH, \
             tc.tile_pool(name="bigN", bufs=1) as bigN, \
             tc.tile_pool(name="bigA", bufs=1) as bigA, \
             tc.tile_pool(name="bigAH", bufs=1) as bigAH, \
             tc.tile_pool(name="upslp", bufs=3) as upslp, \
             tc.tile_pool(name="dwslp", bufs=4) as dwslp, \
             tc.tile_pool(name="psb", bufs=8, space="PSUM") as psb:

            identb = cst.tile([P, P], bf16)
            nc.sync.dma_start(out=identb[:], in_=idb_e.ap())
            wv = cst.tile([P, NTT], f32)
            nc.sync.dma_start(out=wv[:], in_=wv_e.ap())
            al1 = cst.tile([P, 1], f32)
            nc.sync.dma_start(out=al1[:], in_=al1_e.ap())
            upbF = cst.tile([P, MF], f32)
            nc.sync.dma_start(out=upbF[:], in_=upbF_e.ap())
            upbH = cst.tile([P, MF], f32)
            nc.sync.dma_start(out=upbH[:], in_=upbH_e.ap())
            upbI = cst.tile([P, MF], f32)
            nc.sync.dma_start(out=upbI[:], in_=upbI_e.ap())
            eps_t = cst.tile([P, 1], f32)
            nc.vector.memset(eps_t[:], EPS)

            def ln_coeffs(src_stats, alpha, negshift_out, scale_out):
                """src_stats [P,2] (mean,var) -> scale=a*rstd+(1-a),
                shift=-a*mean*rstd (per-partition).  alpha: AP or 1.0."""
                rstd = stp.tile([P, 1], f32, tag="rstd")
                nc.scalar.activation(out=rstd[:], in_=src_stats[:, 1:2],
                                     func=AF.Sqrt, bias=eps_t[:, 0:1])
                nc.vector.reciprocal(out=rstd[:], in_=rstd[:])
                if alpha is None:          # alpha == 1: pure LN
                    nc.vector.tensor_copy(out=scale_out[:], in_=rstd[:])
                    nc.vector.tensor_tensor(out=negshift_out[:],
                                            in0=src_stats[:, 0:1], in1=rstd[:],
                                            op=ALU.mult)
                    nc.vector.tensor_scalar(out=negshift_out[:], in0=negshift_out[:],
                                            scalar1=-1.0, scalar2=None, op0=ALU.mult)
                    return
                nc.vector.tensor_scalar(out=scale_out[:], in0=rstd[:],
                                        scalar1=alpha, scalar2=None, op0=ALU.mult)
                one_m = stp.tile([P, 1], f32, tag="onem")
                nc.vector.tensor_scalar(out=one_m[:], in0=alpha, scalar1=-1.0,
                                        scalar2=1.0, op0=ALU.mult, op1=ALU.add)
                nc.vector.tensor_tensor(out=scale_out[:], in0=scale_out[:],
                                        in1=one_m[:], op=ALU.add)
                nc.vector.tensor_tensor(out=negshift_out[:], in0=src_stats[:, 0:1],
                                        in1=rstd[:], op=ALU.mult)
                nc.vector.tensor_scalar(out=negshift_out[:], in0=negshift_out[:],
                                        scalar1=-1.0, scalar2=None, op0=ALU.mult)
                nc.vector.tensor_tensor(out=negshift_out[:], in0=negshift_out[:],
                                        in1=alpha, op=ALU.mult)

            # ---------- tokens (pre-normalized, transposed) ----------
            nxT = bigN.tile([P, KH, TOT], bf16, tag="nxT")
            nc.sync.dma_start(out=nxT[:, :, FB:HB], in_=nxt_e.ap()[:, :, FB:HB])

            # ---------- up: h1T = gelu(upW.T @ nxT + upb), all 3 streams ----------
            h1T = bigH.tile([P, MF, TOT], bf16, tag="h1T")
            UP_GRPS = [(0, FB, NTF * P // 2, upbF), (0, FB + NTF * P // 2,
                        NTF * P // 2, upbF), (1, HB, NTH * P, upbH),
                       (2, IB, NTI * P, upbI)]
            UP_WES = [upwF_e, upwH_e, upwI_e]
            for m in range(MF):
                us = upslp.tile([P, 3, KH, P], bf16, tag="uslab")
                for si in range(3):
                    nc.sync.dma_start(out=us[:, si], in_=UP_WES[si].ap()[m])
                if m == 0:
                    nc.sync.dma_start(out=nxT[:, :, HB:IB], in_=nxt_e.ap()[:, :, HB:IB])
                    nc.sync.dma_start(out=nxT[:, :, IB:TOT], in_=nxt_e.ap()[:, :, IB:TOT])
                ps_g = [psb.tile([P, G], f32, space="PSUM", tag="acc",
                                 name=f"upps_{m}_{gi}")
                        for gi, (si, c0, G, upb) in enumerate(UP_GRPS)]
                for k in range(KH):
                    for gi, (si, c0, G, upb) in enumerate(UP_GRPS):
                        nc.tensor.matmul(out=ps_g[gi][:], lhsT=us[:, si, k, :],
                                         rhs=nxT[:, k, c0:c0 + G],
                                         start=(k == 0), stop=(k == KH - 1))
                for gi, (si, c0, G, upb) in enumerate(UP_GRPS):
                    nc.scalar.activation(out=h1T[:, m, c0:c0 + G], in_=ps_g[gi][:],
                                         func=AF.Gelu, bias=upb[:, m:m + 1])

            # ---------- M1f: Af = h1F @ W1f + b1f  (SBUF, bf16) + stats ----------
            AbufF = bigA.tile([P, NTF, F], bf16, tag="AbufF")
            stA = [stp.tile([P, NF, nc.vector.BN_STATS_DIM], f32, tag=f"stA{t}",
                            name=f"stA_{t}") for t in range(NTF)]
            for n in range(NF):
                ps_list = [psb.tile([P, 512], f32, space="PSUM", tag="acc",
                                    name=f"accf_{n}_{i}") for i in range(NTF)]
                for k in range(MF):
                    wslab = slp.tile([P, 512], bf16, tag="wslab")
                    nc.sync.dma_start(out=wslab[:], in_=w1f_e.ap()[n, k])
                    for t in range(NTF):
                        nc.tensor.matmul(out=ps_list[t][:],
                                         lhsT=h1T[:, k, FB + t * P:FB + (t + 1) * P],
                                         rhs=wslab[:], start=(k == 0), stop=(k == MF - 1))
                for t in range(NTF):
                    nc.vector.tensor_copy(out=AbufF[:, t, n * 512:(n + 1) * 512],
                                          in_=ps_list[t][:])
                for t in range(NTF):
                    nc.vector.bn_stats(out=stA[t][:, n, :],
                                       in_=AbufF[:, t, n * 512:(n + 1) * 512])

            # ---------- M1h: Ah = gelu(h1H @ W1h + b1h)  [F2 wide] ----------
            AbufH = bigAH.tile([P, NTH, F2], bf16, tag="AbufH")
            for n in range(N4):
                ps_list = [psb.tile([P, 512], f32, space="PSUM", tag="acc",
                                    name=f"acch_{n}_{i}") for i in range(NTH)]
                for k in range(MF):
                    wslab = slp.tile([P, 512], bf16, tag="wslab")
                    nc.sync.dma_start(out=wslab[:], in_=w1h_e.ap()[n, k])
                    for t in range(NTH):
                        nc.tensor.matmul(out=ps_list[t][:],
                                         lhsT=h1T[:, k, HB + t * P:HB + (t + 1) * P],
                                         rhs=wslab[:], start=(k == 0), stop=(k == MF - 1))
                for t in range(NTH):
                    nc.scalar.activation(out=AbufH[:, t, n * 512:(n + 1) * 512],
                                         in_=ps_list[t][:], func=AF.Gelu)

            # ---------- sigma1f: B = gelu(LN-blend(Af)); h2_F = B.T ----------
            for t in range(NTF):
                mvA = stp.tile([P, nc.vector.BN_AGGR_DIM], f32, tag="mvA")
                nc.vector.bn_aggr(out=mvA[:], in_=stA[t][:])
                sc1 = stp.tile([P, 1], f32, tag="sc1")
                sh1 = stp.tile([P, 1], f32, tag="sh1")
                ln_coeffs(mvA, al1[:, 0:1], sh1, sc1)
                for s in range(2):
                    nc.scalar.activation(out=AbufF[:, t, s * 2048:(s + 1) * 2048],
                                         in_=AbufF[:, t, s * 2048:(s + 1) * 2048],
                                         func=AF.Gelu, bias=sh1[:, 0:1],
                                         scale=sc1[:, 0:1])
                for k in range(MF):
                    tpb = psb.tile([P, P], bf16, space="PSUM", tag="acc", name=f"tpb_{t}_{k}")
                    nc.tensor.transpose(out=tpb[:],
                                        in_=AbufF[:, t, k * P:(k + 1) * P],
                                        identity=identb[:])
                    if k % 2 == 0:
                        nc.vector.tensor_copy(out=h1T[:, k, FB + t * P:FB + (t + 1) * P],
                                              in_=tpb[:])
                    else:
                        nc.scalar.activation(out=h1T[:, k, FB + t * P:FB + (t + 1) * P],
                                             in_=tpb[:], func=AF.Copy)

            # ---------- CT = Ah.T ----------
            CT = bigN.tile([P, K2, NTH * P], bf16, tag="nxT")  # reuse nxT slot
            for t in range(NTH):
                for kk in range(K2):
                    tp = psb.tile([P, P], bf16, space="PSUM", tag="acc", name=f"tpc_{t}_{kk}")
                    nc.tensor.transpose(out=tp[:],
                                        in_=AbufH[:, t, kk * P:(kk + 1) * P],
                                        identity=identb[:])
                    if kk % 2 == 0:
                        nc.vector.tensor_copy(out=CT[:, kk, t * P:(t + 1) * P], in_=tp[:])
                    else:
                        nc.scalar.activation(out=CT[:, kk, t * P:(t + 1) * P],
                                             in_=tp[:], func=AF.Copy)

            # ---------- M2: R = Ah @ W2h + b2h  + stats ----------
            RbufH = bigA.tile([P, NTH, F], bf16, tag="AbufF")   # reuse AbufF slot
            st2 = [stp.tile([P, NF, nc.vector.BN_STATS_DIM], f32, tag=f"st2{t}",
                            name=f"st2_{t}") for t in range(NTH)]
            for n in range(NF):
                ps_list = [psb.tile([P, 512], f32, space="PSUM", tag="acc",
                                    name=f"acc2_{n}_{i}") for i in range(NTH)]
                for kk in range(K2):
                    wslab = slp.tile([P, 512], bf16, tag="wslab")
                    nc.sync.dma_start(out=wslab[:], in_=w2h_e.ap()[n, kk])
                    for t in range(NTH):
                        nc.tensor.matmul(out=ps_list[t][:],
                                         lhsT=CT[:, kk, t * P:(t + 1) * P],
                                         rhs=wslab[:], start=(kk == 0), stop=(kk == K2 - 1))
                for t in range(NTH):
                    nc.scalar.activation(out=RbufH[:, t, n * 512:(n + 1) * 512],
                                         in_=ps_list[t][:], func=AF.Copy)
                    nc.vector.bn_stats(out=st2[t][:, n, :],
                                       in_=RbufH[:, t, n * 512:(n + 1) * 512])

            # ---------- down F (h2 ready after sigma1f) ----------
            def down_pass(base, ntx, dw_e):
                for n in range(ND):
                    ps_list = [psb.tile([P, 512], f32, space="PSUM", tag="acc",
                                        name=f"accd_{base}_{n}_{i}") for i in range(ntx)]
                    for k in range(MF):
                        dslab = slp.tile([P, 512], bf16, tag="wslab")
                        nc.sync.dma_start(out=dslab[:], in_=dw_e.ap()[n, k])
                        for t in range(ntx):
                            nc.tensor.matmul(out=ps_list[t][:],
                                             lhsT=h1T[:, k, base + t * P:base + (t + 1) * P],
                                             rhs=dslab[:], start=(k == 0), stop=(k == MF - 1))
                    for t in range(ntx):
                        slot = base // P + t
                        yv = sb.tile([P, 512], f32, tag="yv")
                        nc.scalar.activation(out=yv[:], in_=ps_list[t][:], func=AF.Copy,
                                             scale=wv[:, slot:slot + 1])
                        nc.sync.dma_start(out=y_e.ap()[slot * P:(slot + 1) * P,
                                                       n * 512:(n + 1) * 512],
                                          in_=yv[:])

            def down_pass2(base, ntx, dw_e):
                ps = [[psb.tile([P, 512], f32, space="PSUM", tag="acc",
                                name=f"accd2_{base}_{t}_{n}") for n in range(ND)]
                      for t in range(ntx)]
                for k in range(MF):
                    dslab = dwslp.tile([P, H], bf16, tag="dwslab")
                    nc.sync.dma_start(out=dslab[:], in_=dw_e.ap()[k])
                    for t in range(ntx):
                        for n in range(ND):
                            nc.tensor.matmul(out=ps[t][n][:],
                                             lhsT=h1T[:, k, base + t * P:base + (t + 1) * P],
                                             rhs=dslab[:, n * 512:(n + 1) * 512],
                                             start=(k == 0), stop=(k == MF - 1))
                for t in range(ntx):
                    slot = base // P + t
                    for n in range(ND):
                        yv = sb.tile([P, 512], f32, tag="yv")
                        nc.scalar.activation(out=yv[:], in_=ps[t][n][:], func=AF.Copy,
                                             scale=wv[:, slot:slot + 1])
                        nc.sync.dma_start(out=y_e.ap()[slot * P:(slot + 1) * P,
                                                       n * 512:(n + 1) * 512],
                                          in_=yv[:])

            down_pass(FB, NTF, dwF_e)
            down_pass2(IB, NTI, dwI_e)

            # ---------- sigma2: h2_H = LN(R).T ----------
            for t in range(NTH):
                mv2 = stp.tile([P, nc.vector.BN_AGGR_DIM], f32, tag="mv2")
                nc.vector.bn_aggr(out=mv2[:], in_=st2[t][:])
                sc2 = stp.tile([P, 1], f32, tag="sc2")
                sh2 = stp.tile([P, 1], f32, tag="sh2")
                ln_coeffs(mv2, None, sh2, sc2)      # alpha = 1 (pure LN)
                for s in range(2):
                    nc.vector.tensor_scalar(out=RbufH[:, t, s * 2048:(s + 1) * 2048],
                                            in0=RbufH[:, t, s * 2048:(s + 1) * 2048],
                                            scalar1=sc2[:, 0:1], scalar2=sh2[:, 0:1],
                                            op0=ALU.mult, op1=ALU.add)
                for k in range(MF):
                    tpr = psb.tile([P, P], bf16, space="PSUM", tag="acc", name=f"tpr_{t}_{k}")
                    nc.tensor.transpose(out=tpr[:],
                                        in_=RbufH[:, t, k * P:(k + 1) * P],
                                        identity=identb[:])
                    if k % 2 == 0:
                        nc.vector.tensor_copy(out=h1T[:, k, HB + t * P:HB + (t + 1) * P],
                                              in_=tpr[:])
                    else:
                        nc.scalar.activation(out=h1T[:, k, HB + t * P:HB + (t + 1) * P],
                                             in_=tpr[:], func=AF.Copy)

            # ---------- down H ----------
            down_pass2(HB, NTH, dwH_e)

    nc.finalize()
    return nc


def _routing(x_flat, ln_g, ln_b, router_w):
    """Bit-exact replication of the reference router on jax CPU.
    Returns (combine weights, top-k mask, normalized tokens)."""
    import jax
    import jax.numpy as jnp
    cpu = jax.devices("cpu")[0]
    with jax.default_device(cpu):
        x = jnp.asarray(np.asarray(x_flat))
        g = jnp.asarray(np.asarray(ln_g))
        b = jnp.asarray(np.asarray(ln_b))
        rw = jnp.asarray(np.asarray(router_w))
        m = jnp.mean(x, axis=-1, keepdims=True)
        v = jnp.var(x, axis=-1, keepdims=True)
        nx = (x - m) / jnp.sqrt(v + 1e-5) * g + b
        logits = nx @ rw
        probs = jax.nn.softmax(logits, axis=-1)
        _, idx = jax.lax.top_k(probs, K)
        mask = jnp.sum(jax.nn.one_hot(idx, probs.shape[-1], dtype=probs.dtype), axis=1)
        w = probs * mask
        w = w / jnp.sum(w, axis=-1, keepdims=True)
        return np.asarray(w), np.asarray(mask), np.asarray(nx)


def _col128(vec, n):
    return np.ascontiguousarray(vec.reshape(n, P).T)


def _swz(wmat, nf, kk):
    # [kk*128, nf*512] -> [nf, kk, 128, 512]
    r = wmat.reshape(kk, P, nf, 512)
    return np.ascontiguousarray(r.transpose(2, 0, 1, 3)).astype(BF16)


def _chunks(lst, size):
    return [lst[i:i + size] for i in range(0, len(lst), size)]


def kernel(**inputs):
    from concourse.bass_utils import run_bass_kernel_spmd

    global _CACHED_NC
    x = np.asarray(inputs["hidden_states"], np.float32)
    x_flat = x.reshape(T, H)
    w_all, mask, nx = _routing(x_flat, inputs["ln_g"], inputs["ln_b"],
                               inputs["router_w"])

    up_W = np.asarray(inputs["up_W"], np.float32)
    up_b = np.asarray(inputs["up_b"], np.float32)
    down_W = np.asarray(inputs["down_W"], np.float32)
    down_b = np.asarray(inputs["down_b"], np.float32)
    spec0_W = np.asarray(inputs["spec0_W"], np.float32)
    spec0_b = np.asarray(inputs["spec0_b"], np.float32)
    spec1a_W = np.asarray(inputs["spec1a_W"], np.float32)
    spec1a_b = np.asarray(inputs["spec1a_b"], np.float32)
    spec1b_W = np.asarray(inputs["spec1b_W"], np.float32)
    spec1b_b = np.asarray(inputs["spec1b_b"], np.float32)
    spec2_W = np.asarray(inputs["spec2_W"], np.float32)
    spec2_b = np.asarray(inputs["spec2_b"], np.float32)
    ln0_g = np.asarray(inputs["ln0_g"], np.float32)
    ln0_b = np.asarray(inputs["ln0_b"], np.float32)
    ln1_g = np.asarray(inputs["ln1_g"], np.float32)
    ln1_b = np.asarray(inputs["ln1_b"], np.float32)
    assert np.all(ln0_g == 1) and np.all(ln0_b == 0), "ln0 affine folding not implemented"
    assert np.all(ln1_g == 1) and np.all(ln1_b == 0), "ln1 affine folding not implemented"
    for bias in (up_b, down_b, spec0_b, spec1a_b, spec1b_b, spec2_b):
        assert np.all(bias == 0), "nonzero biases not supported by this kernel"

    # --- per-expert padded token-tile lists ---
    tok = {e: np.nonzero(mask[:, e] > 0)[0] for e in range(E)}
    tiles = {}          # e -> list of arrays of 128 token ids (padded with -1)
    for e in range(E):
        ids = tok[e]
        ntile = (len(ids) + P - 1) // P
        pad = np.full(ntile * P, -1, np.int64)
        pad[:len(ids)] = ids
        tiles[e] = [pad[i * P:(i + 1) * P] for i in range(ntile)]

    # --- assign tile chunks to cores: one expert per stream per core ---
    def assign(experts, cap):
        per_core = [None] * 8       # (expert, [tiles]) or None
        c = 0
        for e in experts:
            for ch in _chunks(tiles[e], cap):
                assert c < 8, "stream assignment overflow"
                per_core[c] = (e, ch)
                c += 1
        return per_core

    f_asgn = assign(F_EXPERTS, NTF)
    h_asgn = assign(H_EXPERTS, NTH)
    i_asgn = assign(I_EXPERTS, NTI)
    assert all(a is not None for a in f_asgn), "F stream must fill all cores"

    zeros_w1f = np.zeros((NF, MF, P, 512), BF16)
    zeros_w1h = np.zeros((N4, MF, P, 512), BF16)
    zeros_w2h = np.zeros((NF, K2, P, 512), BF16)
    zeros_dw = np.zeros((ND, MF, P, 512), BF16)
    zeros_dw2 = np.zeros((MF, P, H), BF16)
    zeros_upw = np.zeros((MF, P, KH, P), BF16)

    def upw_swz(e):
        return np.ascontiguousarray(
            up_W[e].reshape(KH, P, MF, P).transpose(2, 1, 0, 3)).astype(BF16)

    in_maps = []
    scatter = []        # per core: (slot_token_ids [TOT], valid mask [TOT])
    for c in range(8):
        slot_ids = np.zeros(TOT, np.int64)
        valid = np.zeros(TOT, bool)
        wvv = np.zeros(TOT, np.float32)

        def fill(asgn, base, nslots):
            if asgn is None:
                return None
            e, ch = asgn
            for ti, tile_ids in enumerate(ch):
                s0 = base + ti * P
                real = tile_ids >= 0
                slot_ids[s0:s0 + P][real] = tile_ids[real]
                valid[s0:s0 + P] = real
                wvv[s0:s0 + P][real] = w_all[tile_ids[real], e]
            return e

        eF = fill(f_asgn[c], FB, NTF)
        eH = fill(h_asgn[c], HB, NTH)
        eI = fill(i_asgn[c], IB, NTI)

        nxg = np.zeros((TOT, H), np.float32)
        nxg[valid] = nx[slot_ids[valid]]
        nxt = np.ascontiguousarray(
            nxg.reshape(TOT, KH, P).transpose(2, 1, 0)).astype(BF16)

        jF = 0 if eF < 4 else 1
        if eF in (0, 4):
            w1f, b1f, a1 = spec0_W[jF], spec0_b[jF], 1.0
        else:
            w1f, b1f, a1 = spec2_W[jF], spec2_b[jF], 0.0

        im = {
            "nxt": nxt,
            "idb": np.eye(P, dtype=np.float32).astype(BF16),
            "wv": _col128(wvv, NTT),
            "al1": np.full((P, 1), a1, np.float32),
            "upwF": upw_swz(eF),
            "upbF": _col128(up_b[eF], MF),
            "w1f": _swz(w1f, NF, MF),
            "dwF": _swz(down_W[eF], ND, MF),
        }
        if eH is not None:
            jH = 0 if eH < 4 else 1
            im["upwH"] = upw_swz(eH)
            im["upbH"] = _col128(up_b[eH], MF)
            im["w1h"] = _swz(spec1a_W[jH], N4, MF)
            im["w2h"] = _swz(spec1b_W[jH], NF, K2)
            im["dwH"] = np.ascontiguousarray(down_W[eH].reshape(MF, P, H)).astype(BF16)
        else:
            im["upwH"] = zeros_upw
            im["upbH"] = np.zeros((P, MF), np.float32)
            im["w1h"] = zeros_w1h
            im["w2h"] = zeros_w2h
            im["dwH"] = zeros_dw2
        if eI is not None:
            im["upwI"] = upw_swz(eI)
            im["upbI"] = _col128(up_b[eI], MF)
            im["dwI"] = np.ascontiguousarray(down_W[eI].reshape(MF, P, H)).astype(BF16)
        else:
            im["upwI"] = zeros_upw
            im["upbI"] = np.zeros((P, MF), np.float32)
            im["dwI"] = zeros_dw2

        in_maps.append(im)
        scatter.append((slot_ids, valid))

    if _CACHED_NC is None:
        _CACHED_NC = _build_nc()
    trace = os.environ.get("BASS_MOE_TRACE") == "1"
    res = run_bass_kernel_spmd(_CACHED_NC, in_maps, list(range(8)), trace=trace)
    global LAST_RES
    LAST_RES = res

    y = x_flat.copy()
    for c in range(8):
        slot_ids, valid = scatter[c]
        out = res.results[c]["y"]
        np.add.at(y, slot_ids[valid], out[valid])
    return y.reshape(B, S, H)
